# revision 8
# baseline (speedup 1.0000x reference)
"""Multi-head graph attention (GAT-style) Trainium2 Bass kernel.

Full-input contract: kernel(**inputs) takes the complete arrays, shards
batch-wise across 8 NeuronCores (2 batches each), and gathers the output.

Math per batch b, head h (KD=16 head dim):
  Q = h @ Wq_h, K = h @ Wk_h, V = h @ Wv_h            [N, 16]
  compatT[m, n] = (K Q^T)[m, n]                        [N, N] (transposed)
  p = exp(0.25 * compatT) * adjT                       (mask after exp; exact:
      masked entries are exactly 0, matching softmax(-inf) * adj)
  headsT[v, n] = (V'.T @ p)  with V' = [V | 1]         -> row 16 = denominator
  out[n, :] = sum_h (headsT_h / denom_h).T @ Wout_h + h[n, :]

No max-subtraction: logits are O(+-15) so exp stays in fp32 range and
softmax is shift-invariant.

Host-side prep (layout only): h transposed to hT, adj transposed + cast to
bf16 (0/1 exact), weights reshaped. All FLOPs run on device.
"""

import numpy as np
import ml_dtypes
from contextlib import ExitStack

import concourse.bass as bass
import concourse.mybir as mybir
import concourse.tile as tile
from concourse.bass_utils import run_bass_kernel_spmd

B, N, E, H, KD = 16, 1024, 128, 8, 16
CORES = 8
BPC = B // CORES  # batches per core
F32 = mybir.dt.float32
BF16 = mybir.dt.bfloat16
NT = 512  # fp32 matmul moving-operand max free dim
MC = N // 128  # number of 128-row chunks of m / n


def build_kernel():
    nc = bass.Bass()
    hT_d = nc.dram_tensor("ht", [BPC, E, N], F32, kind="ExternalInput")
    h_d = nc.dram_tensor("hn", [BPC, N, E], F32, kind="ExternalInput")
    adjt_d = nc.dram_tensor("adjt", [BPC, N, N], BF16, kind="ExternalInput")
    wq_d = nc.dram_tensor("wq", [E, H * KD], F32, kind="ExternalInput")
    wk_d = nc.dram_tensor("wk", [E, H * KD], F32, kind="ExternalInput")
    wv_d = nc.dram_tensor("wv", [E, H * KD], F32, kind="ExternalInput")
    wo_d = nc.dram_tensor("wo", [H * KD, E], F32, kind="ExternalInput")
    out_d = nc.dram_tensor("out", [BPC, N, E], F32, kind="ExternalOutput")

    with ExitStack() as ctx:
        tc = ctx.enter_context(tile.TileContext(nc))
        consts = ctx.enter_context(tc.tile_pool(name="consts", bufs=1))
        io_pool = ctx.enter_context(tc.tile_pool(name="io", bufs=2))
        qk_pool = ctx.enter_context(tc.tile_pool(name="qk", bufs=1))
        v_pool = ctx.enter_context(tc.tile_pool(name="v", bufs=2))
        p_pool = ctx.enter_context(tc.tile_pool(name="p", bufs=2))
        hd_pool = ctx.enter_context(tc.tile_pool(name="hd", bufs=2))
        ps_qkv = ctx.enter_context(tc.tile_pool(name="ps_qkv", bufs=1, space="PSUM"))
        ps_c = ctx.enter_context(tc.tile_pool(name="ps_c", bufs=2, space="PSUM"))
        ps_h = ctx.enter_context(tc.tile_pool(name="ps_h", bufs=1, space="PSUM"))
        dram = ctx.enter_context(tc.tile_pool(name="dram", bufs=2, space="DRAM"))

        wq_sb = consts.tile([E, H * KD], F32, tag="wq")
        wk_sb = consts.tile([E, H * KD], F32, tag="wk")
        wv_sb = consts.tile([E, H * KD], F32, tag="wv")
        wo_sb = consts.tile([H * KD, E], F32, tag="wo")
        nc.sync.dma_start(out=wq_sb, in_=wq_d[:, :])
        nc.sync.dma_start(out=wk_sb, in_=wk_d[:, :])
        nc.sync.dma_start(out=wv_sb, in_=wv_d[:, :])
        nc.sync.dma_start(out=wo_sb, in_=wo_d[:, :])

        for b in range(BPC):
            hT_sb = io_pool.tile([E, N], F32, tag="ht")
            nc.sync.dma_start(out=hT_sb, in_=hT_d[b, :, :])
            h_sb = io_pool.tile([128, MC, E], F32, tag="hn")
            nc.sync.dma_start(
                out=h_sb, in_=h_d[b].rearrange("(c p) e -> p c e", p=128)
            )
            adjT_sb = io_pool.tile([128, MC, N], BF16, tag="adj")
            nc.sync.dma_start(
                out=adjT_sb, in_=adjt_d[b].rearrange("(c p) n -> p c n", p=128)
            )

            # ---- projections (all heads packed: partition = h*16+k) ----
            def project(w_sb, tag):
                ps = ps_qkv.tile([H * KD, N], F32, tag="qkv")
                for nt in range(N // NT):
                    nc.tensor.matmul(
                        out=ps[:, nt * NT : (nt + 1) * NT],
                        lhsT=w_sb,
                        rhs=hT_sb[:, nt * NT : (nt + 1) * NT],
                        start=True,
                        stop=True,
                    )
                packed = qk_pool.tile([H * KD, N], F32, tag=f"{tag}pk")
                nc.vector.tensor_copy(out=packed, in_=ps)
                # shift each head's 16 rows down to base partition 0
                per_head = qk_pool.tile([KD, H, N], F32, tag=f"{tag}ph")
                for hi in range(H):
                    nc.sync.dma_start(
                        out=per_head[:, hi, :],
                        in_=packed[hi * KD : (hi + 1) * KD, :],
                    )
                return per_head

            qT_sb = project(wq_sb, "q")
            kT_sb = project(wk_sb, "k")

            # V natural [m, h, 17] in bf16, col 16 = ones (denominator trick)
            v_nat = []
            for mc in range(MC):
                v_ps = ps_qkv.tile([128, H * KD], F32, tag="qkv")
                nc.tensor.matmul(
                    out=v_ps,
                    lhsT=hT_sb[:, mc * 128 : (mc + 1) * 128],
                    rhs=wv_sb,
                    start=True,
                    stop=True,
                )
                vt = v_pool.tile([128, H, KD + 1], BF16, tag=f"v{mc}")
                nc.vector.tensor_copy(
                    out=vt[:, :, 0:KD], in_=v_ps.rearrange("p (h k) -> p h k", k=KD)
                )
                nc.vector.memset(vt[:, :, KD : KD + 1], 1.0)
                v_nat.append(vt)

            # ---- attention per head ----
            headsTu = hd_pool.tile([128, N], F32, tag="hu")  # rows h*16+v
            denoms = hd_pool.tile([H, N], F32, tag="den")
            for hi in range(H):
                heads_ps = ps_h.tile([KD + 1, N], F32, tag="heads")
                for mc in range(MC):
                    c_ps = ps_c.tile([128, N], F32, tag="compat")
                    for nt in range(N // NT):
                        nc.tensor.matmul(
                            out=c_ps[:, nt * NT : (nt + 1) * NT],
                            lhsT=kT_sb[:, hi, mc * 128 : (mc + 1) * 128],
                            rhs=qT_sb[:, hi, nt * NT : (nt + 1) * NT],
                            start=True,
                            stop=True,
                        )
                    pT = p_pool.tile([128, N], BF16, tag="pt")
                    nc.scalar.activation(
                        out=pT,
                        in_=c_ps,
                        func=mybir.ActivationFunctionType.Exp,
                        scale=0.25,
                    )
                    pm = p_pool.tile([128, N], BF16, tag="pm")
                    nc.vector.tensor_mul(pm, pT, adjT_sb[:, mc, :])
                    for nt in range(N // NT):
                        nc.tensor.matmul(
                            out=heads_ps[:, nt * NT : (nt + 1) * NT],
                            lhsT=v_nat[mc][:, hi, :],
                            rhs=pm[:, nt * NT : (nt + 1) * NT],
                            start=(mc == 0),
                            stop=(mc == MC - 1),
                        )
                hu_h = hd_pool.tile([KD + 1, N], F32, tag="huh")
                nc.vector.tensor_copy(out=hu_h, in_=heads_ps)
                nc.sync.dma_start(
                    out=headsTu[hi * KD : (hi + 1) * KD, :], in_=hu_h[0:KD, :]
                )
                nc.sync.dma_start(
                    out=denoms[hi : hi + 1, :], in_=hu_h[KD : KD + 1, :]
                )

            # ---- normalize: headsTn = headsTu * bcast(1/denom) ----
            recip = hd_pool.tile([H, N], F32, tag="rec")
            nc.vector.reciprocal(out=recip, in_=denoms)
            rec_dram = dram.tile([H, N], F32, tag="recd")
            nc.sync.dma_start(out=rec_dram, in_=recip)
            recip_bc = hd_pool.tile([128, N], F32, tag="recbc")
            for hi in range(H):
                src = rec_dram[hi : hi + 1, :]
                bc = bass.AP(
                    tensor=src.tensor,
                    offset=src.offset,
                    ap=[[0, KD]] + list(src.ap[1:]),
                )
                nc.gpsimd.dma_start(
                    out=recip_bc[hi * KD : (hi + 1) * KD, :], in_=bc
                )
            headsTn = hd_pool.tile([128, N], F32, tag="hnorm")
            nc.vector.tensor_mul(headsTn, headsTu, recip_bc)

            # ---- output: out[cc*128:, :] = headsTn[:, cc].T @ Wout + h ----
            for cc in range(MC):
                o_ps = ps_qkv.tile([128, E], F32, tag="qkv")
                nc.tensor.matmul(
                    out=o_ps,
                    lhsT=headsTn[:, cc * 128 : (cc + 1) * 128],
                    rhs=wo_sb,
                    start=True,
                    stop=True,
                )
                ob = hd_pool.tile([128, E], F32, tag="ob")
                nc.vector.tensor_add(ob, o_ps, h_sb[:, cc, :])
                nc.sync.dma_start(
                    out=out_d[b, cc * 128 : (cc + 1) * 128, :], in_=ob
                )
    return nc


def _split_multi_waits(nc):
    """walrus codegen in this container allows only one sync-wait per
    instruction; hoist extra waits onto preceding same-engine nops."""
    import copy
    import bass_rust

    tmpl_nc = bass.Bass()
    tmpls = {}
    for en in ["vector", "scalar", "tensor", "gpsimd", "sync"]:
        ins = getattr(tmpl_nc, en).nop().ins
        tmpls[str(ins.engine)] = ins

    uid = [0]
    for fn in nc.m.functions:
        for bb in fn.blocks:
            out = []
            for ins in bb.instructions:
                si = ins.sync_info
                waits = list(si.on_wait) if si is not None else []
                if len(waits) > 1:
                    for w in waits[:-1]:
                        nop = copy.deepcopy(tmpls[str(ins.engine)])
                        uid[0] += 1
                        nop.name = f"I-splitw-{uid[0]}"
                        nop.sync_info = bass_rust.SyncInfo(
                            on_wait=[w], on_update=[]
                        )
                        out.append(nop)
                    ins.sync_info = bass_rust.SyncInfo(
                        on_wait=[waits[-1]], on_update=list(si.on_update)
                    )
                out.append(ins)
            bb.instructions = out
    return nc


_cache = {}


def _get_nc():
    if "nc" not in _cache:
        _cache["nc"] = _split_multi_waits(build_kernel())
    return _cache["nc"]


def kernel(h, adj_c, W_query, W_key, W_val, W_out, trace=False):
    h = np.asarray(h, np.float32)
    adj = np.asarray(adj_c)
    hT = np.ascontiguousarray(h.transpose(0, 2, 1))  # [B, E, N]
    adjT = np.ascontiguousarray(
        adj.transpose(0, 2, 1).astype(ml_dtypes.bfloat16)
    )  # [B, N(m), N(n)] bf16
    wq = np.ascontiguousarray(
        np.asarray(W_query, np.float32).transpose(1, 0, 2).reshape(E, H * KD)
    )
    wk = np.ascontiguousarray(
        np.asarray(W_key, np.float32).transpose(1, 0, 2).reshape(E, H * KD)
    )
    wv = np.ascontiguousarray(
        np.asarray(W_val, np.float32).transpose(1, 0, 2).reshape(E, H * KD)
    )
    wo = np.ascontiguousarray(np.asarray(W_out, np.float32).reshape(H * KD, E))

    nc = _get_nc()
    in_maps = []
    for c in range(CORES):
        s = slice(c * BPC, (c + 1) * BPC)
        in_maps.append(
            {
                "ht": np.ascontiguousarray(hT[s]),
                "hn": np.ascontiguousarray(h[s]),
                "adjt": np.ascontiguousarray(adjT[s]),
                "wq": wq,
                "wk": wk,
                "wv": wv,
                "wo": wo,
            }
        )
    res = run_bass_kernel_spmd(nc, in_maps, core_ids=list(range(CORES)), trace=trace)
    out = np.concatenate([r["out"] for r in res.results], axis=0)
    if trace:
        return out, res
    return out


# revision 12
# speedup vs baseline: 1.2055x; 1.2055x over previous
"""Multi-head graph attention (GAT-style) Trainium2 Bass kernel.

Full-input contract: kernel(**inputs) takes the complete arrays, shards
batch-wise across 8 NeuronCores (2 batches each), and gathers the output.

Math per batch b, head h (KD=16 head dim):
  Q = h @ Wq_h, K = h @ Wk_h, V = h @ Wv_h            [N, 16]
  compatT[m, n] = (K Q^T)[m, n]                        [N, N] (transposed)
  p = exp(0.25 * compatT) * adjT                       (mask after exp; exact:
      masked entries are exactly 0, matching softmax(-inf) * adj)
  headsT[v, n] = (V'.T @ p)  with V' = [V | 1]         -> row 16 = denominator
  out[n, :] = sum_h (headsT_h / denom_h).T @ Wout_h + h[n, :]

No max-subtraction: logits are O(+-15) so exp stays in fp32 range and
softmax is shift-invariant.

Host-side prep (layout only): h transposed to hT, adj transposed + cast to
bf16 (0/1 exact), weights reshaped. All FLOPs run on device.
"""

import numpy as np
import ml_dtypes
from contextlib import ExitStack

import concourse.bass as bass
import concourse.mybir as mybir
import concourse.tile as tile
from concourse.bass_utils import run_bass_kernel_spmd

B, N, E, H, KD = 16, 1024, 128, 8, 16
CORES = 8
BPC = B // CORES  # batches per core
F32 = mybir.dt.float32
BF16 = mybir.dt.bfloat16
NT = 512  # fp32 matmul moving-operand max free dim
MC = N // 128  # number of 128-row chunks of m / n


def build_kernel():
    nc = bass.Bass()
    hT_d = nc.dram_tensor("ht", [BPC, E, N], F32, kind="ExternalInput")
    h_d = nc.dram_tensor("hn", [BPC, N, E], F32, kind="ExternalInput")
    adjt_d = nc.dram_tensor("adjt", [BPC, N, N], BF16, kind="ExternalInput")
    wq_d = nc.dram_tensor("wq", [E, H * KD], F32, kind="ExternalInput")
    wk_d = nc.dram_tensor("wk", [E, H * KD], F32, kind="ExternalInput")
    wv_d = nc.dram_tensor("wv", [E, H * KD], F32, kind="ExternalInput")
    wo_d = nc.dram_tensor("wo", [H * KD, E], F32, kind="ExternalInput")
    out_d = nc.dram_tensor("out", [BPC, N, E], F32, kind="ExternalOutput")

    with ExitStack() as ctx:
        tc = ctx.enter_context(tile.TileContext(nc))
        consts = ctx.enter_context(tc.tile_pool(name="consts", bufs=1))
        io_pool = ctx.enter_context(tc.tile_pool(name="io", bufs=2))
        qk_pool = ctx.enter_context(tc.tile_pool(name="qk", bufs=1))
        v_pool = ctx.enter_context(tc.tile_pool(name="v", bufs=2))
        p_pool = ctx.enter_context(tc.tile_pool(name="p", bufs=1))
        pt_pool = ctx.enter_context(tc.tile_pool(name="pt", bufs=2))
        ob_pool = ctx.enter_context(tc.tile_pool(name="ob", bufs=2))
        hd_pool = ctx.enter_context(tc.tile_pool(name="hd", bufs=1))
        ps_c = ctx.enter_context(tc.tile_pool(name="ps_c", bufs=3, space="PSUM"))
        ps_h = ctx.enter_context(tc.tile_pool(name="ps_h", bufs=1, space="PSUM"))
        dram = ctx.enter_context(tc.tile_pool(name="dram", bufs=2, space="DRAM"))

        wq_sb = consts.tile([E, H * KD], F32, tag="wq")
        wk_sb = consts.tile([E, H * KD], F32, tag="wk")
        wv_sb = consts.tile([E, H * KD], F32, tag="wv")
        wo_sb = consts.tile([H * KD, E], F32, tag="wo")
        nc.sync.dma_start(out=wq_sb, in_=wq_d[:, :])
        nc.sync.dma_start(out=wk_sb, in_=wk_d[:, :])
        nc.sync.dma_start(out=wv_sb, in_=wv_d[:, :])
        nc.sync.dma_start(out=wo_sb, in_=wo_d[:, :])

        for b in range(BPC):
            hT_sb = io_pool.tile([E, N], F32, tag="ht")
            nc.sync.dma_start(out=hT_sb, in_=hT_d[b, :, :])
            h_sb = io_pool.tile([128, MC, E], F32, tag="hn")
            nc.sync.dma_start(
                out=h_sb, in_=h_d[b].rearrange("(c p) e -> p c e", p=128)
            )
            adjT_sb = io_pool.tile([128, MC, N], BF16, tag="adj")
            nc.sync.dma_start(
                out=adjT_sb, in_=adjt_d[b].rearrange("(c p) n -> p c n", p=128)
            )

            # ---- projections (all heads packed: partition = h*16+k) ----
            def project(w_sb, tag):
                ps = ps_c.tile([H * KD, N], F32, tag="compat")
                for nt in range(N // NT):
                    nc.tensor.matmul(
                        out=ps[:, nt * NT : (nt + 1) * NT],
                        lhsT=w_sb,
                        rhs=hT_sb[:, nt * NT : (nt + 1) * NT],
                        start=True,
                        stop=True,
                    )
                packed = qk_pool.tile([H * KD, N], F32, tag=f"{tag}pk")
                nc.vector.tensor_copy(out=packed, in_=ps)
                # shift each head's 16 rows down to base partition 0
                per_head = qk_pool.tile([KD, H, N], F32, tag=f"{tag}ph")
                for hi in range(H):
                    nc.sync.dma_start(
                        out=per_head[:, hi, :],
                        in_=packed[hi * KD : (hi + 1) * KD, :],
                    )
                return per_head

            qT_sb = project(wq_sb, "q")
            kT_sb = project(wk_sb, "k")

            # V natural [m, h, 17] in bf16, col 16 = ones (denominator trick)
            v_nat = []
            for mc in range(MC):
                v_ps = ps_c.tile([128, H * KD], F32, tag="compat")
                nc.tensor.matmul(
                    out=v_ps,
                    lhsT=hT_sb[:, mc * 128 : (mc + 1) * 128],
                    rhs=wv_sb,
                    start=True,
                    stop=True,
                )
                vt = v_pool.tile([128, H, KD + 1], BF16, tag=f"v{mc}")
                nc.vector.tensor_copy(
                    out=vt[:, :, 0:KD], in_=v_ps.rearrange("p (h k) -> p h k", k=KD)
                )
                nc.vector.memset(vt[:, :, KD : KD + 1], 1.0)
                v_nat.append(vt)

            # ---- attention, 4-head groups (PV col-tiled into one PSUM) ----
            # Emission order per head: all 8 compat matmuls back-to-back,
            # then the 8 PV matmuls — keeps PE streaming while ACT/DVE chew
            # on exp/mask of earlier m-chunks (avoids HAM re-throttle).
            headsTu = hd_pool.tile([128, N], F32, tag="hu")  # rows h*16+v
            denoms = hd_pool.tile([H, N], F32, tag="den")
            for g in range(H // 4):
                hp4 = ps_h.tile([128, N], F32, tag="heads")  # head j @ 32j
                for jj in range(4):
                    hi = g * 4 + jj
                    pms = []
                    for mc in range(MC):
                        c_ps = ps_c.tile([128, N], F32, tag="compat")
                        for nt in range(N // NT):
                            nc.tensor.matmul(
                                out=c_ps[:, nt * NT : (nt + 1) * NT],
                                lhsT=kT_sb[:, hi, mc * 128 : (mc + 1) * 128],
                                rhs=qT_sb[:, hi, nt * NT : (nt + 1) * NT],
                                start=True,
                                stop=True,
                            )
                        pT = pt_pool.tile([128, N], BF16, tag="pt")
                        nc.scalar.activation(
                            out=pT,
                            in_=c_ps,
                            func=mybir.ActivationFunctionType.Exp,
                            scale=0.25,
                        )
                        pm = p_pool.tile([128, N], BF16, tag=f"pm{mc}")
                        nc.vector.tensor_mul(pm, pT, adjT_sb[:, mc, :])
                        pms.append(pm)
                    for mc in range(MC):
                        for nt in range(N // NT):
                            nc.tensor.matmul(
                                out=hp4[
                                    32 * jj : 32 * jj + KD + 1,
                                    nt * NT : (nt + 1) * NT,
                                ],
                                lhsT=v_nat[mc][:, hi, :],
                                rhs=pms[mc][:, nt * NT : (nt + 1) * NT],
                                start=(mc == 0),
                                stop=(mc == MC - 1),
                                tile_position=(0, 32 * jj),
                            )
                hu4 = hd_pool.tile([128, N], F32, tag="huh")
                nc.vector.tensor_copy(out=hu4, in_=hp4)
                for jj in range(4):
                    hi = g * 4 + jj
                    nc.sync.dma_start(
                        out=headsTu[hi * KD : (hi + 1) * KD, :],
                        in_=hu4[32 * jj : 32 * jj + KD, :],
                    )
                    nc.sync.dma_start(
                        out=denoms[hi : hi + 1, :],
                        in_=hu4[32 * jj + KD : 32 * jj + KD + 1, :],
                    )

            # ---- normalize: headsTn = headsTu * bcast(1/denom) ----
            recip = hd_pool.tile([H, N], F32, tag="rec")
            nc.vector.reciprocal(out=recip, in_=denoms)
            rec_dram = dram.tile([H, N], F32, tag="recd")
            nc.sync.dma_start(out=rec_dram, in_=recip)
            recip_bc = hd_pool.tile([128, N], F32, tag="recbc")
            for hi in range(H):
                src = rec_dram[hi : hi + 1, :]
                bc = bass.AP(
                    tensor=src.tensor,
                    offset=src.offset,
                    ap=[[0, KD]] + list(src.ap[1:]),
                )
                nc.gpsimd.dma_start(
                    out=recip_bc[hi * KD : (hi + 1) * KD, :], in_=bc
                )
            headsTn = hd_pool.tile([128, N], F32, tag="hnorm")
            nc.vector.tensor_mul(headsTn, headsTu, recip_bc)

            # ---- output: out[cc*128:, :] = headsTn[:, cc].T @ Wout + h ----
            for cc in range(MC):
                o_ps = ps_c.tile([128, E], F32, tag="compat")
                nc.tensor.matmul(
                    out=o_ps,
                    lhsT=headsTn[:, cc * 128 : (cc + 1) * 128],
                    rhs=wo_sb,
                    start=True,
                    stop=True,
                )
                ob = ob_pool.tile([128, E], F32, tag="ob")
                nc.vector.tensor_add(ob, o_ps, h_sb[:, cc, :])
                nc.sync.dma_start(
                    out=out_d[b, cc * 128 : (cc + 1) * 128, :], in_=ob
                )
    return nc


def _split_multi_waits(nc):
    """walrus codegen in this container allows only one sync-wait per
    instruction; hoist extra waits onto preceding same-engine nops."""
    import copy
    import bass_rust

    tmpl_nc = bass.Bass()
    tmpls = {}
    for en in ["vector", "scalar", "tensor", "gpsimd", "sync"]:
        ins = getattr(tmpl_nc, en).nop().ins
        tmpls[str(ins.engine)] = ins

    uid = [0]
    for fn in nc.m.functions:
        for bb in fn.blocks:
            out = []
            for ins in bb.instructions:
                si = ins.sync_info
                waits = list(si.on_wait) if si is not None else []
                if len(waits) > 1:
                    for w in waits[:-1]:
                        nop = copy.deepcopy(tmpls[str(ins.engine)])
                        uid[0] += 1
                        nop.name = f"I-splitw-{uid[0]}"
                        nop.sync_info = bass_rust.SyncInfo(
                            on_wait=[w], on_update=[]
                        )
                        out.append(nop)
                    ins.sync_info = bass_rust.SyncInfo(
                        on_wait=[waits[-1]], on_update=list(si.on_update)
                    )
                out.append(ins)
            bb.instructions = out
    return nc


_cache = {}


def _get_nc():
    if "nc" not in _cache:
        _cache["nc"] = _split_multi_waits(build_kernel())
    return _cache["nc"]


def kernel(h, adj_c, W_query, W_key, W_val, W_out, trace=False):
    h = np.asarray(h, np.float32)
    adj = np.asarray(adj_c)
    hT = np.ascontiguousarray(h.transpose(0, 2, 1))  # [B, E, N]
    adjT = np.ascontiguousarray(
        adj.transpose(0, 2, 1).astype(ml_dtypes.bfloat16)
    )  # [B, N(m), N(n)] bf16
    wq = np.ascontiguousarray(
        np.asarray(W_query, np.float32).transpose(1, 0, 2).reshape(E, H * KD)
    )
    wk = np.ascontiguousarray(
        np.asarray(W_key, np.float32).transpose(1, 0, 2).reshape(E, H * KD)
    )
    wv = np.ascontiguousarray(
        np.asarray(W_val, np.float32).transpose(1, 0, 2).reshape(E, H * KD)
    )
    wo = np.ascontiguousarray(np.asarray(W_out, np.float32).reshape(H * KD, E))

    nc = _get_nc()
    in_maps = []
    for c in range(CORES):
        s = slice(c * BPC, (c + 1) * BPC)
        in_maps.append(
            {
                "ht": np.ascontiguousarray(hT[s]),
                "hn": np.ascontiguousarray(h[s]),
                "adjt": np.ascontiguousarray(adjT[s]),
                "wq": wq,
                "wk": wk,
                "wv": wv,
                "wo": wo,
            }
        )
    res = run_bass_kernel_spmd(nc, in_maps, core_ids=list(range(CORES)), trace=trace)
    out = np.concatenate([r["out"] for r in res.results], axis=0)
    if trace:
        return out, res
    return out


# revision 14
# speedup vs baseline: 2.0110x; 1.6682x over previous
"""Multi-head graph attention (GAT-style) Trainium2 Bass kernel.

Full-input contract: kernel(**inputs) takes the complete arrays, shards
batch-wise across 8 NeuronCores (2 batches each), and gathers the output.

Math per batch b, head h (KD=16 head dim):
  Q = h @ Wq_h, K = h @ Wk_h, V = h @ Wv_h            [N, 16]
  compatT[m, n] = (K Q^T)[m, n]                        [N, N] (transposed)
  p = exp(0.25 * compatT) * adjT                       (mask after exp; exact:
      masked entries are exactly 0, matching softmax(-inf) * adj)
  headsT[v, n] = (V'.T @ p)  with V' = [V | 1]         -> row 16 = denominator
  out[n, :] = sum_h (headsT_h / denom_h).T @ Wout_h + h[n, :]

No max-subtraction: logits are O(+-15) so exp stays in fp32 range and
softmax is shift-invariant.

Host-side prep (layout only): h transposed to hT, adj transposed + cast to
bf16 (0/1 exact), weights reshaped. All FLOPs run on device.
"""

import numpy as np
import ml_dtypes
from contextlib import ExitStack

import concourse.bass as bass
import concourse.mybir as mybir
import concourse.tile as tile
from concourse.bass_utils import run_bass_kernel_spmd

B, N, E, H, KD = 16, 1024, 128, 8, 16
CORES = 8
BPC = B // CORES  # batches per core
F32 = mybir.dt.float32
BF16 = mybir.dt.bfloat16
NT = 512  # fp32 matmul moving-operand max free dim
MC = N // 128  # number of 128-row chunks of m / n


def build_kernel():
    nc = bass.Bass()
    hT_d = nc.dram_tensor("ht", [BPC, E, N], F32, kind="ExternalInput")
    h_d = nc.dram_tensor("hn", [BPC, N, E], F32, kind="ExternalInput")
    adjt_d = nc.dram_tensor("adjt", [BPC, N, N], BF16, kind="ExternalInput")
    wq_d = nc.dram_tensor("wq", [E, H * KD], F32, kind="ExternalInput")
    wk_d = nc.dram_tensor("wk", [E, H * KD], F32, kind="ExternalInput")
    wv_d = nc.dram_tensor("wv", [E, H * KD], F32, kind="ExternalInput")
    wo_d = nc.dram_tensor("wo", [H * KD, E], F32, kind="ExternalInput")
    out_d = nc.dram_tensor("out", [BPC, N, E], F32, kind="ExternalOutput")

    with ExitStack() as ctx:
        tc = ctx.enter_context(tile.TileContext(nc))
        consts = ctx.enter_context(tc.tile_pool(name="consts", bufs=1))
        io_pool = ctx.enter_context(tc.tile_pool(name="io", bufs=2))
        qk_pool = ctx.enter_context(tc.tile_pool(name="qk", bufs=1))
        v_pool = ctx.enter_context(tc.tile_pool(name="v", bufs=2))
        p_pool = ctx.enter_context(tc.tile_pool(name="p", bufs=1))
        pt_pool = ctx.enter_context(tc.tile_pool(name="pt", bufs=2))
        ob_pool = ctx.enter_context(tc.tile_pool(name="ob", bufs=2))
        hd_pool = ctx.enter_context(tc.tile_pool(name="hd", bufs=1))
        ps_c = ctx.enter_context(tc.tile_pool(name="ps_c", bufs=3, space="PSUM"))
        ps_h = ctx.enter_context(tc.tile_pool(name="ps_h", bufs=1, space="PSUM"))
        dram = ctx.enter_context(tc.tile_pool(name="dram", bufs=2, space="DRAM"))

        wq_sb = consts.tile([E, H * KD], F32, tag="wq")
        wk_sb = consts.tile([E, H * KD], F32, tag="wk")
        wv_sb = consts.tile([E, H * KD], F32, tag="wv")
        wo_sb = consts.tile([H * KD, E], F32, tag="wo")
        nc.sync.dma_start(out=wq_sb, in_=wq_d[:, :])
        nc.sync.dma_start(out=wk_sb, in_=wk_d[:, :])
        nc.sync.dma_start(out=wv_sb, in_=wv_d[:, :])
        nc.sync.dma_start(out=wo_sb, in_=wo_d[:, :])

        for b in range(BPC):
            hT_sb = io_pool.tile([E, N], F32, tag="ht")
            nc.sync.dma_start(out=hT_sb, in_=hT_d[b, :, :])
            h_sb = io_pool.tile([128, MC, E], F32, tag="hn")
            nc.sync.dma_start(
                out=h_sb, in_=h_d[b].rearrange("(c p) e -> p c e", p=128)
            )
            adjT_sb = io_pool.tile([128, MC, N], BF16, tag="adj")
            nc.sync.dma_start(
                out=adjT_sb, in_=adjt_d[b].rearrange("(c p) n -> p c n", p=128)
            )

            # ---- projections (all heads packed: partition = h*16+k) ----
            def project(w_sb, tag):
                ps = ps_c.tile([H * KD, N], F32, tag="compat")
                for nt in range(N // NT):
                    nc.tensor.matmul(
                        out=ps[:, nt * NT : (nt + 1) * NT],
                        lhsT=w_sb,
                        rhs=hT_sb[:, nt * NT : (nt + 1) * NT],
                        start=True,
                        stop=True,
                    )
                packed = qk_pool.tile([H * KD, N], BF16, tag=f"{tag}pk")
                nc.vector.tensor_copy(out=packed, in_=ps)
                # shift each head's 16 rows down to base partition 0
                per_head = qk_pool.tile([KD, H, N], BF16, tag=f"{tag}ph")
                for hi in range(H):
                    nc.sync.dma_start(
                        out=per_head[:, hi, :],
                        in_=packed[hi * KD : (hi + 1) * KD, :],
                    )
                return per_head

            qT_sb = project(wq_sb, "q")
            kT_sb = project(wk_sb, "k")

            # V natural [m, h, 17] in bf16, col 16 = ones (denominator trick)
            v_nat = []
            for mc in range(MC):
                v_ps = ps_c.tile([128, H * KD], F32, tag="compat")
                nc.tensor.matmul(
                    out=v_ps,
                    lhsT=hT_sb[:, mc * 128 : (mc + 1) * 128],
                    rhs=wv_sb,
                    start=True,
                    stop=True,
                )
                vt = v_pool.tile([128, H, KD + 1], BF16, tag=f"v{mc}")
                nc.vector.tensor_copy(
                    out=vt[:, :, 0:KD], in_=v_ps.rearrange("p (h k) -> p h k", k=KD)
                )
                nc.vector.memset(vt[:, :, KD : KD + 1], 1.0)
                v_nat.append(vt)

            # ---- attention, 4-head groups (PV col-tiled into one PSUM) ----
            # Emission order per head: all 8 compat matmuls back-to-back,
            # then the 8 PV matmuls — keeps PE streaming while ACT/DVE chew
            # on exp/mask of earlier m-chunks (avoids HAM re-throttle).
            headsTu = hd_pool.tile([128, N], F32, tag="hu")  # rows h*16+v
            denoms = hd_pool.tile([H, N], F32, tag="den")
            for g in range(H // 4):
                hp4 = ps_h.tile([128, N], F32, tag="heads")  # head j @ 32j
                for jj in range(4):
                    hi = g * 4 + jj
                    pms = []
                    for mc in range(MC):
                        c_ps = ps_c.tile([128, N], F32, tag="compat")
                        for nt in range(N // NT):
                            nc.tensor.matmul(
                                out=c_ps[:, nt * NT : (nt + 1) * NT],
                                lhsT=kT_sb[:, hi, mc * 128 : (mc + 1) * 128],
                                rhs=qT_sb[:, hi, nt * NT : (nt + 1) * NT],
                                start=True,
                                stop=True,
                            )
                        pT = pt_pool.tile([128, N], BF16, tag="pt")
                        nc.scalar.activation(
                            out=pT,
                            in_=c_ps,
                            func=mybir.ActivationFunctionType.Exp,
                            scale=0.25,
                        )
                        pm = p_pool.tile([128, N], BF16, tag=f"pm{mc}")
                        nc.vector.tensor_mul(pm, pT, adjT_sb[:, mc, :])
                        pms.append(pm)
                    for mc in range(MC):
                        for nt in range(N // NT):
                            nc.tensor.matmul(
                                out=hp4[
                                    32 * jj : 32 * jj + KD + 1,
                                    nt * NT : (nt + 1) * NT,
                                ],
                                lhsT=v_nat[mc][:, hi, :],
                                rhs=pms[mc][:, nt * NT : (nt + 1) * NT],
                                start=(mc == 0),
                                stop=(mc == MC - 1),
                                tile_position=(0, 32 * jj),
                            )
                hu4 = hd_pool.tile([128, N], F32, tag="huh")
                nc.vector.tensor_copy(out=hu4, in_=hp4)
                for jj in range(4):
                    hi = g * 4 + jj
                    nc.sync.dma_start(
                        out=headsTu[hi * KD : (hi + 1) * KD, :],
                        in_=hu4[32 * jj : 32 * jj + KD, :],
                    )
                    nc.sync.dma_start(
                        out=denoms[hi : hi + 1, :],
                        in_=hu4[32 * jj + KD : 32 * jj + KD + 1, :],
                    )

            # ---- normalize: headsTn = headsTu * bcast(1/denom) ----
            recip = hd_pool.tile([H, N], F32, tag="rec")
            nc.vector.reciprocal(out=recip, in_=denoms)
            rec_dram = dram.tile([H, N], F32, tag="recd")
            nc.sync.dma_start(out=rec_dram, in_=recip)
            recip_bc = hd_pool.tile([128, N], F32, tag="recbc")
            for hi in range(H):
                src = rec_dram[hi : hi + 1, :]
                bc = bass.AP(
                    tensor=src.tensor,
                    offset=src.offset,
                    ap=[[0, KD]] + list(src.ap[1:]),
                )
                nc.gpsimd.dma_start(
                    out=recip_bc[hi * KD : (hi + 1) * KD, :], in_=bc
                )
            headsTn = hd_pool.tile([128, N], F32, tag="hnorm")
            nc.vector.tensor_mul(headsTn, headsTu, recip_bc)

            # ---- output: out[cc*128:, :] = headsTn[:, cc].T @ Wout + h ----
            for cc in range(MC):
                o_ps = ps_c.tile([128, E], F32, tag="compat")
                nc.tensor.matmul(
                    out=o_ps,
                    lhsT=headsTn[:, cc * 128 : (cc + 1) * 128],
                    rhs=wo_sb,
                    start=True,
                    stop=True,
                )
                ob = ob_pool.tile([128, E], F32, tag="ob")
                nc.vector.tensor_add(ob, o_ps, h_sb[:, cc, :])
                nc.sync.dma_start(
                    out=out_d[b, cc * 128 : (cc + 1) * 128, :], in_=ob
                )
    return nc


def _split_multi_waits(nc):
    """walrus codegen in this container allows only one sync-wait per
    instruction; hoist extra waits onto preceding same-engine nops."""
    import copy
    import bass_rust

    tmpl_nc = bass.Bass()
    tmpls = {}
    for en in ["vector", "scalar", "tensor", "gpsimd", "sync"]:
        ins = getattr(tmpl_nc, en).nop().ins
        tmpls[str(ins.engine)] = ins

    uid = [0]
    for fn in nc.m.functions:
        for bb in fn.blocks:
            out = []
            for ins in bb.instructions:
                si = ins.sync_info
                waits = list(si.on_wait) if si is not None else []
                if len(waits) > 1:
                    for w in waits[:-1]:
                        nop = copy.deepcopy(tmpls[str(ins.engine)])
                        uid[0] += 1
                        nop.name = f"I-splitw-{uid[0]}"
                        nop.sync_info = bass_rust.SyncInfo(
                            on_wait=[w], on_update=[]
                        )
                        out.append(nop)
                    ins.sync_info = bass_rust.SyncInfo(
                        on_wait=[waits[-1]], on_update=list(si.on_update)
                    )
                out.append(ins)
            bb.instructions = out
    return nc


_cache = {}


def _get_nc():
    if "nc" not in _cache:
        _cache["nc"] = _split_multi_waits(build_kernel())
    return _cache["nc"]


def kernel(h, adj_c, W_query, W_key, W_val, W_out, trace=False):
    h = np.asarray(h, np.float32)
    adj = np.asarray(adj_c)
    hT = np.ascontiguousarray(h.transpose(0, 2, 1))  # [B, E, N]
    adjT = np.ascontiguousarray(
        adj.transpose(0, 2, 1).astype(ml_dtypes.bfloat16)
    )  # [B, N(m), N(n)] bf16
    wq = np.ascontiguousarray(
        np.asarray(W_query, np.float32).transpose(1, 0, 2).reshape(E, H * KD)
    )
    wk = np.ascontiguousarray(
        np.asarray(W_key, np.float32).transpose(1, 0, 2).reshape(E, H * KD)
    )
    wv = np.ascontiguousarray(
        np.asarray(W_val, np.float32).transpose(1, 0, 2).reshape(E, H * KD)
    )
    wo = np.ascontiguousarray(np.asarray(W_out, np.float32).reshape(H * KD, E))

    nc = _get_nc()
    in_maps = []
    for c in range(CORES):
        s = slice(c * BPC, (c + 1) * BPC)
        in_maps.append(
            {
                "ht": np.ascontiguousarray(hT[s]),
                "hn": np.ascontiguousarray(h[s]),
                "adjt": np.ascontiguousarray(adjT[s]),
                "wq": wq,
                "wk": wk,
                "wv": wv,
                "wo": wo,
            }
        )
    res = run_bass_kernel_spmd(nc, in_maps, core_ids=list(range(CORES)), trace=trace)
    out = np.concatenate([r["out"] for r in res.results], axis=0)
    if trace:
        return out, res
    return out


# revision 16
# speedup vs baseline: 2.0401x; 1.0145x over previous
"""Multi-head graph attention (GAT-style) Trainium2 Bass kernel.

Full-input contract: kernel(**inputs) takes the complete arrays, shards
batch-wise across 8 NeuronCores (2 batches each), and gathers the output.

Math per batch b, head h (KD=16 head dim):
  Q = h @ Wq_h, K = h @ Wk_h, V = h @ Wv_h            [N, 16]
  compatT[m, n] = (K Q^T)[m, n]                        [N, N] (transposed)
  p = exp(0.25 * compatT) * adjT                       (mask after exp; exact:
      masked entries are exactly 0, matching softmax(-inf) * adj)
  headsT[v, n] = (V'.T @ p)  with V' = [V | 1]         -> row 16 = denominator
  out[n, :] = sum_h (headsT_h / denom_h).T @ Wout_h + h[n, :]

No max-subtraction: logits are O(+-15) so exp stays in fp32 range and
softmax is shift-invariant.

Host-side prep (layout only): h transposed to hT, adj transposed + cast to
bf16 (0/1 exact), weights reshaped. All FLOPs run on device.
"""

import numpy as np
import ml_dtypes
from contextlib import ExitStack

import concourse.bass as bass
import concourse.mybir as mybir
import concourse.tile as tile
from concourse.bass_utils import run_bass_kernel_spmd

B, N, E, H, KD = 16, 1024, 128, 8, 16
CORES = 8
BPC = B // CORES  # batches per core
F32 = mybir.dt.float32
BF16 = mybir.dt.bfloat16
NT = 512  # fp32 matmul moving-operand max free dim
MC = N // 128  # number of 128-row chunks of m / n


def build_kernel():
    nc = bass.Bass()
    hT_d = nc.dram_tensor("ht", [BPC, E, N], F32, kind="ExternalInput")
    h_d = nc.dram_tensor("hn", [BPC, N, E], F32, kind="ExternalInput")
    adjt_d = nc.dram_tensor("adjt", [BPC, N, N], BF16, kind="ExternalInput")
    wq_d = nc.dram_tensor("wq", [E, H * KD], F32, kind="ExternalInput")
    wk_d = nc.dram_tensor("wk", [E, H * KD], F32, kind="ExternalInput")
    wv_d = nc.dram_tensor("wv", [E, H * KD], F32, kind="ExternalInput")
    wo_d = nc.dram_tensor("wo", [H * KD, E], F32, kind="ExternalInput")
    out_d = nc.dram_tensor("out", [BPC, N, E], F32, kind="ExternalOutput")

    with ExitStack() as ctx:
        tc = ctx.enter_context(tile.TileContext(nc))
        consts = ctx.enter_context(tc.tile_pool(name="consts", bufs=1))
        io_pool = ctx.enter_context(tc.tile_pool(name="io", bufs=2))
        qk_pool = ctx.enter_context(tc.tile_pool(name="qk", bufs=1))
        v_pool = ctx.enter_context(tc.tile_pool(name="v", bufs=2))
        p_pool = ctx.enter_context(tc.tile_pool(name="p", bufs=2))
        pt_pool = ctx.enter_context(tc.tile_pool(name="pt", bufs=4))
        ob_pool = ctx.enter_context(tc.tile_pool(name="ob", bufs=2))
        hd_pool = ctx.enter_context(tc.tile_pool(name="hd", bufs=1))
        ps_c = ctx.enter_context(tc.tile_pool(name="ps_c", bufs=3, space="PSUM"))
        ps_h = ctx.enter_context(tc.tile_pool(name="ps_h", bufs=1, space="PSUM"))
        dram = ctx.enter_context(tc.tile_pool(name="dram", bufs=2, space="DRAM"))

        wq_sb = consts.tile([E, H * KD], F32, tag="wq")
        wk_sb = consts.tile([E, H * KD], F32, tag="wk")
        wv_sb = consts.tile([E, H * KD], F32, tag="wv")
        wo_sb = consts.tile([H * KD, E], F32, tag="wo")
        nc.sync.dma_start(out=wq_sb, in_=wq_d[:, :])
        nc.sync.dma_start(out=wk_sb, in_=wk_d[:, :])
        nc.sync.dma_start(out=wv_sb, in_=wv_d[:, :])
        nc.sync.dma_start(out=wo_sb, in_=wo_d[:, :])

        for b in range(BPC):
            hT_sb = io_pool.tile([E, N], F32, tag="ht")
            nc.sync.dma_start(out=hT_sb, in_=hT_d[b, :, :])
            h_sb = io_pool.tile([128, MC, E], F32, tag="hn")
            nc.sync.dma_start(
                out=h_sb, in_=h_d[b].rearrange("(c p) e -> p c e", p=128)
            )
            adjT_sb = io_pool.tile([128, MC, N], BF16, tag="adj")
            nc.sync.dma_start(
                out=adjT_sb, in_=adjt_d[b].rearrange("(c p) n -> p c n", p=128)
            )

            # ---- projections (all heads packed: partition = h*16+k) ----
            def project(w_sb, tag):
                ps = ps_c.tile([H * KD, N], F32, tag="compat")
                for nt in range(N // NT):
                    nc.tensor.matmul(
                        out=ps[:, nt * NT : (nt + 1) * NT],
                        lhsT=w_sb,
                        rhs=hT_sb[:, nt * NT : (nt + 1) * NT],
                        start=True,
                        stop=True,
                    )
                packed = qk_pool.tile([H * KD, N], BF16, tag=f"{tag}pk")
                nc.vector.tensor_copy(out=packed, in_=ps)
                # shift each head's 16 rows down to base partition 0
                per_head = qk_pool.tile([KD, H, N], BF16, tag=f"{tag}ph")
                for hi in range(H):
                    nc.sync.dma_start(
                        out=per_head[:, hi, :],
                        in_=packed[hi * KD : (hi + 1) * KD, :],
                    )
                return per_head

            qT_sb = project(wq_sb, "q")
            kT_sb = project(wk_sb, "k")

            # V natural [m, h, 17] in bf16, col 16 = ones (denominator trick)
            v_nat = []
            for mc in range(MC):
                v_ps = ps_c.tile([128, H * KD], F32, tag="compat")
                nc.tensor.matmul(
                    out=v_ps,
                    lhsT=hT_sb[:, mc * 128 : (mc + 1) * 128],
                    rhs=wv_sb,
                    start=True,
                    stop=True,
                )
                vt = v_pool.tile([128, H, KD + 1], BF16, tag=f"v{mc}")
                nc.vector.tensor_copy(
                    out=vt[:, :, 0:KD], in_=v_ps.rearrange("p (h k) -> p h k", k=KD)
                )
                nc.vector.memset(vt[:, :, KD : KD + 1], 1.0)
                v_nat.append(vt)

            # ---- attention, 4-head groups (PV col-tiled into one PSUM) ----
            # Emission order per head: all 8 compat matmuls back-to-back,
            # then the 8 PV matmuls — keeps PE streaming while ACT/DVE chew
            # on exp/mask of earlier m-chunks (avoids HAM re-throttle).
            headsTu = hd_pool.tile([128, N], F32, tag="hu")  # rows h*16+v
            denoms = hd_pool.tile([H, N], F32, tag="den")
            for g in range(H // 4):
                hp4 = ps_h.tile([128, N], F32, tag="heads")  # head j @ 32j

                def emit_pv(pms, mc):
                    # 4 heads' PV interleaved across PE column groups so the
                    # MMs run concurrently (tile_position col packing).
                    for nt in range(N // NT):
                        for jj in range(4):
                            nc.tensor.matmul(
                                out=hp4[
                                    32 * jj : 32 * jj + KD + 1,
                                    nt * NT : (nt + 1) * NT,
                                ],
                                lhsT=v_nat[mc][:, g * 4 + jj, :],
                                rhs=pms[jj][:, nt * NT : (nt + 1) * NT],
                                start=(mc == 0),
                                stop=(mc == MC - 1),
                                tile_position=(0, 32 * jj),
                            )

                prev = None
                for mc in range(MC):
                    cur = []
                    for jj in range(4):
                        hi = g * 4 + jj
                        c_ps = ps_c.tile([128, N], F32, tag="compat")
                        for nt in range(N // NT):
                            nc.tensor.matmul(
                                out=c_ps[:, nt * NT : (nt + 1) * NT],
                                lhsT=kT_sb[:, hi, mc * 128 : (mc + 1) * 128],
                                rhs=qT_sb[:, hi, nt * NT : (nt + 1) * NT],
                                start=True,
                                stop=True,
                            )
                        pT = pt_pool.tile([128, N], BF16, tag="pt")
                        nc.scalar.activation(
                            out=pT,
                            in_=c_ps,
                            func=mybir.ActivationFunctionType.Exp,
                            scale=0.25,
                        )
                        pm = p_pool.tile([128, N], BF16, tag=f"pm{jj}")
                        nc.vector.tensor_mul(pm, pT, adjT_sb[:, mc, :])
                        cur.append(pm)
                    if prev is not None:
                        emit_pv(prev, mc - 1)
                    prev = cur
                emit_pv(prev, MC - 1)
                hu4 = hd_pool.tile([128, N], F32, tag="huh")
                nc.vector.tensor_copy(out=hu4, in_=hp4)
                for jj in range(4):
                    hi = g * 4 + jj
                    nc.sync.dma_start(
                        out=headsTu[hi * KD : (hi + 1) * KD, :],
                        in_=hu4[32 * jj : 32 * jj + KD, :],
                    )
                    nc.sync.dma_start(
                        out=denoms[hi : hi + 1, :],
                        in_=hu4[32 * jj + KD : 32 * jj + KD + 1, :],
                    )

            # ---- normalize: headsTn = headsTu * bcast(1/denom) ----
            recip = hd_pool.tile([H, N], F32, tag="rec")
            nc.vector.reciprocal(out=recip, in_=denoms)
            rec_dram = dram.tile([H, N], F32, tag="recd")
            nc.sync.dma_start(out=rec_dram, in_=recip)
            recip_bc = hd_pool.tile([128, N], F32, tag="recbc")
            for hi in range(H):
                src = rec_dram[hi : hi + 1, :]
                bc = bass.AP(
                    tensor=src.tensor,
                    offset=src.offset,
                    ap=[[0, KD]] + list(src.ap[1:]),
                )
                nc.gpsimd.dma_start(
                    out=recip_bc[hi * KD : (hi + 1) * KD, :], in_=bc
                )
            headsTn = hd_pool.tile([128, N], F32, tag="hnorm")
            nc.vector.tensor_mul(headsTn, headsTu, recip_bc)

            # ---- output: out[cc*128:, :] = headsTn[:, cc].T @ Wout + h ----
            for cc in range(MC):
                o_ps = ps_c.tile([128, E], F32, tag="compat")
                nc.tensor.matmul(
                    out=o_ps,
                    lhsT=headsTn[:, cc * 128 : (cc + 1) * 128],
                    rhs=wo_sb,
                    start=True,
                    stop=True,
                )
                ob = ob_pool.tile([128, E], F32, tag="ob")
                nc.vector.tensor_add(ob, o_ps, h_sb[:, cc, :])
                nc.sync.dma_start(
                    out=out_d[b, cc * 128 : (cc + 1) * 128, :], in_=ob
                )
    return nc


def _split_multi_waits(nc):
    """walrus codegen in this container allows only one sync-wait per
    instruction; hoist extra waits onto preceding same-engine nops."""
    import copy
    import bass_rust

    tmpl_nc = bass.Bass()
    tmpls = {}
    for en in ["vector", "scalar", "tensor", "gpsimd", "sync"]:
        ins = getattr(tmpl_nc, en).nop().ins
        tmpls[str(ins.engine)] = ins

    uid = [0]
    for fn in nc.m.functions:
        for bb in fn.blocks:
            out = []
            for ins in bb.instructions:
                si = ins.sync_info
                waits = list(si.on_wait) if si is not None else []
                if len(waits) > 1:
                    for w in waits[:-1]:
                        nop = copy.deepcopy(tmpls[str(ins.engine)])
                        uid[0] += 1
                        nop.name = f"I-splitw-{uid[0]}"
                        nop.sync_info = bass_rust.SyncInfo(
                            on_wait=[w], on_update=[]
                        )
                        out.append(nop)
                    ins.sync_info = bass_rust.SyncInfo(
                        on_wait=[waits[-1]], on_update=list(si.on_update)
                    )
                out.append(ins)
            bb.instructions = out
    return nc


_cache = {}


def _get_nc():
    if "nc" not in _cache:
        _cache["nc"] = _split_multi_waits(build_kernel())
    return _cache["nc"]


def kernel(h, adj_c, W_query, W_key, W_val, W_out, trace=False):
    h = np.asarray(h, np.float32)
    adj = np.asarray(adj_c)
    hT = np.ascontiguousarray(h.transpose(0, 2, 1))  # [B, E, N]
    adjT = np.ascontiguousarray(
        adj.transpose(0, 2, 1).astype(ml_dtypes.bfloat16)
    )  # [B, N(m), N(n)] bf16
    wq = np.ascontiguousarray(
        np.asarray(W_query, np.float32).transpose(1, 0, 2).reshape(E, H * KD)
    )
    wk = np.ascontiguousarray(
        np.asarray(W_key, np.float32).transpose(1, 0, 2).reshape(E, H * KD)
    )
    wv = np.ascontiguousarray(
        np.asarray(W_val, np.float32).transpose(1, 0, 2).reshape(E, H * KD)
    )
    wo = np.ascontiguousarray(np.asarray(W_out, np.float32).reshape(H * KD, E))

    nc = _get_nc()
    in_maps = []
    for c in range(CORES):
        s = slice(c * BPC, (c + 1) * BPC)
        in_maps.append(
            {
                "ht": np.ascontiguousarray(hT[s]),
                "hn": np.ascontiguousarray(h[s]),
                "adjt": np.ascontiguousarray(adjT[s]),
                "wq": wq,
                "wk": wk,
                "wv": wv,
                "wo": wo,
            }
        )
    res = run_bass_kernel_spmd(nc, in_maps, core_ids=list(range(CORES)), trace=trace)
    out = np.concatenate([r["out"] for r in res.results], axis=0)
    if trace:
        return out, res
    return out
